# revision 28
# baseline (speedup 1.0000x reference)
"""DepthAwareConv2d Trainium2 kernel (bf16 v12).

Math: the reference's depth-modulated im2col GEMM is exactly
    out = conv2d(x * depth, weight, stride=1, pad=1) + bias
(depth broadcasts over channels; unfold(x)*unfold(depth) = unfold(x*depth)).

Sharding (8 cores): data-parallel over N (4 images) x spatial-parallel over
image row halves.  Core cid handles n = cid//2, row half = cid%2 (output rows
[0,64) or [64,128)), computing all 256 output channels for its half.  The
host ships each core its 64 input rows plus one halo/zero row on each side
(66 rows total), so the device program is identical on every core (SPMD) and
no collectives are needed.

The matmul train runs at the bf16 PE floor (217ns per 512-col matmul,
~63us for the 306 matmuls); everything else is edge-engineering driven by
trace measurements:
  * HW-measured HWDGE queue rates: Sync ~170GB/s, ScalarE ~95GB/s (it
    shares bandwidth with the activation-table load); startup is also
    HBM-contended by all 8 cores streaming at once.  The input schedule is
    laid out against those rates in consumption order (queues are FIFO):
    rows 0-11 ship as separate x / depth transfers so the first multiply
    only waits for two parallel 128KB queue heads; rows 12-65 ship packed
    [x|depth] per row (one DMA + ONE completion semaphore per chunk),
    distributed across both queues so each stays ahead of the train.
  * the bias DMA (128 partitions x 8B = 128 tiny descriptors, ~1-2us of
    queue time) is kept behind the early chunks.
  * NWARM short (128-col) zero matmuls run back-to-back from right after
    the engine barrier until the first real data lands: the PE's HAM
    activity window stays continuously busy, so the clock un-gates 4/8 ->
    8/8 ~3.4us after the barrier instead of mid-train (a gap during the
    ramp was observed to cost ~2-3us of half-clock matmuls); each adds only
    ~110ns of queue delay once real data is ready.
  * output DMA triggers ride the Sync HWDGE (ScalarE keeps draining PSUM
    via ACTIVATE); both ob passes run back to back (ob=1 weights are far
    off the critical path) and start/end with a 2-row block (early first
    matmul, short final drain); the final block drains on the idle DVE.

Per-core device kernel:
  1. DMA x/depth rows into SBUF; DVE-multiply into column-padded bf16 ypad
     (128 part, 66 x 130).
  2. Shift-conv: per output-row block and 128-wide out-channel half, 9
     accumulating bf16 matmuls (stationary = 128x128 weight tap, moving =
     shifted row window, free dim <= 512 = one PSUM bank) into one PSUM bank.
  3. ScalarE Identity(+bias) PSUM->SBUF (bf16), DMA out via Sync HWDGE.
"""

import ml_dtypes
import numpy as np

import concourse.bass as bass
import concourse.mybir as mybir
import concourse.tile as tile
from concourse import bacc
from concourse.bass_utils import run_bass_kernel_spmd

N, C, O, H, W = 4, 128, 256, 128, 128
HSH = H // 2  # output rows per core
HIN = HSH + 2  # input rows per core incl. halo/zero rows
NSPLIT = 12  # rows shipped as separate x / depth (the startup window)
NCORES = 8
F32 = mybir.dt.float32
BF16 = mybir.dt.bfloat16
ACT_IDENT = mybir.ActivationFunctionType.Identity
NPBF16 = ml_dtypes.bfloat16

RB = 4  # output rows per matmul tile (free dim RB*W = 512, one PSUM bank)
# packed [x|depth] chunks for rows 12-65: (row0, row1, queue); boundaries
# 0 mod 4; issue (= queue FIFO) order is consumption order per queue
PACKED = (
    (12, 16, "S"),
    (16, 20, "S"),
    (20, 28, "S"),
    (28, 36, "A"),  # the slow ScalarE queue gets one mid chunk + wt1
    (36, 44, "S"),
    (44, 56, "S"),
    (56, 66, "S"),
)
# ~36 x ~107ns covers the barrier -> first-data window (first real matmul
# lands ~11.3-11.7us with the drip-fed weight taps) with no PE gap: a >1us
# gap during the ramp was seen to delay the HAM un-throttle mid-train,
# costing ~2.5us of half-clock matmuls.
NWARM = 36

_CACHE = {}


def build_nc():
    nc = bacc.Bacc("TRN2", target_bir_lowering=False, debug=False, num_devices=NCORES)
    xs = nc.declare_dram_parameter("xs", [C, NSPLIT, W], BF16, isOutput=False)
    dep = nc.declare_dram_parameter("dep", [C, NSPLIT, W], BF16, isOutput=False)
    xd = nc.declare_dram_parameter(
        "xd", [C, HIN - NSPLIT, 2, W], BF16, isOutput=False
    )
    wt = nc.declare_dram_parameter("wt", [C, 2, 9, O // 2], BF16, isOutput=False)
    bb = nc.declare_dram_parameter("bb", [O // 2, 2], F32, isOutput=False)
    out = nc.declare_dram_parameter("out", [O, HSH, W], BF16, isOutput=True)

    with tile.TileContext(nc) as tc:
        with (
            tc.tile_pool(name="big", bufs=1) as big,
            tc.tile_pool(name="wp", bufs=1) as wp,
            tc.tile_pool(name="ch", bufs=7) as chp,
            tc.tile_pool(name="op", bufs=4) as op,
            tc.tile_pool(name="pp", bufs=8, space="PSUM") as pp,
        ):
            ypad = big.tile([C, HIN, W + 2], BF16)
            wsb = wp.tile([C, 2, 9, O // 2], BF16)
            bsb = wp.tile([O // 2, 2], F32)  # bsb[p, ob] = bias[ob*128 + p]
            ztile = wp.tile([C, 128], BF16)

            # zeros: warmup operand + ypad column borders.  GpSimd is idle
            # otherwise and starts right after the engine barrier, so the
            # warm-up matmuls (which read ztile) issue as early as possible.
            nc.gpsimd.memset(ztile, 0.0)
            nc.gpsimd.memset(ypad[:, :, 0], 0.0)
            nc.gpsimd.memset(ypad[:, :, W + 1], 0.0)

            warm = pp.tile([O // 2, 128], F32, tag="ps")
            for _ in range(NWARM):
                nc.tensor.matmul(warm, ztile, ztile, start=True, stop=True)

            split_tiles = {}
            packed_tiles = {}

            def split_dma(r0, r1):
                xb = chp.tile([C, 8, W], BF16, tag="xb", name=f"xb{r0}")
                db = chp.tile([C, 8, W], BF16, tag="db", name=f"db{r0}")
                nc.sync.dma_start(out=xb[:, : r1 - r0], in_=xs[:, r0:r1])
                nc.scalar.dma_start(out=db[:, : r1 - r0], in_=dep[:, r0:r1])
                split_tiles[r0] = (xb, db, r0, r1)

            def packed_dma(pi):
                r0, r1, q = PACKED[pi]
                xb = chp.tile([C, 12, 2, W], BF16, tag="xd", name=f"xd{r0}")
                eng = nc.sync if q == "S" else nc.scalar
                eng.dma_start(
                    out=xb[:, : r1 - r0], in_=xd[:, r0 - NSPLIT : r1 - NSPLIT]
                )
                packed_tiles[pi] = xb

            def mul_rows(r0, r1):
                # 4-aligned multiply blocks keep Tile's (quantized)
                # range-overlap check from dragging in a writer one byte
                # past the true read range of a conv block.
                if r0 < NSPLIT:
                    xb, db, base, end = next(
                        v for v in split_tiles.values() if v[2] <= r0 < v[3]
                    )
                    lo = r0 - base
                    in0 = xb[:, lo : lo + (r1 - r0)]
                    in1 = db[:, lo : lo + (r1 - r0)]
                else:
                    pi = next(
                        i for i, (a, b, _) in enumerate(PACKED) if a <= r0 < b
                    )
                    xb = packed_tiles[pi]
                    lo = r0 - PACKED[pi][0]
                    in0 = xb[:, lo : lo + (r1 - r0), 0]
                    in1 = xb[:, lo : lo + (r1 - r0), 1]
                nc.vector.tensor_mul(out=ypad[:, r0:r1, 1 : W + 1], in0=in0, in1=in1)

            # trigger order = consumption order per queue (FIFO).  Sync: x0,
            # then the ob=0 weights drip-fed as three 98KB tap-triples
            # between the x chunks -- block 0 consumes one tap per ~215ns,
            # so tap p only has to beat first_mm + p*215ns, and the first
            # matmul is gated by the 98KB wt[0,:3] instead of the full
            # 294KB half (which measured ~2.5us late).  ScalarE: dep0,
            # dep1, bias, its packed chunk, the ob=1 weights.
            split_dma(0, 4)
            nc.sync.dma_start(out=wsb[:, 0, :3], in_=wt.ap()[:, 0, :3])
            nc.sync.dma_start(out=wsb[:, 0, 3:6], in_=wt.ap()[:, 0, 3:6])
            split_dma(4, 12)
            nc.sync.dma_start(out=wsb[:, 0, 6:], in_=wt.ap()[:, 0, 6:])
            nc.scalar.dma_start(out=bsb, in_=bb.ap())
            mul_rows(0, 4)
            for pi in range(len(PACKED)):
                packed_dma(pi)
            nc.scalar.dma_start(out=wsb[:, 1], in_=wt.ap()[:, 1])
            r = 4
            while r < HIN:
                r1 = min(r + 4, HIN)
                mul_rows(r, r1)
                r = r1

            def conv_block(r0, nrows, ob, name, drain=nc.scalar):
                # one accumulation group: rows [r0, r0+nrows), out-half ob
                ps = pp.tile([O // 2, nrows, W], F32, tag="ps", name=f"ps{name}")
                for p in range(9):
                    i, j = divmod(p, 3)
                    nc.tensor.matmul(
                        ps,
                        wsb[:, ob, p],
                        ypad[:, r0 + i : r0 + i + nrows, j : j + W],
                        start=(p == 0),
                        stop=(p == 8),
                    )
                osb = op.tile([O // 2, nrows, W], BF16, tag="osb", name=f"osb{name}")
                if drain is nc.scalar:
                    nc.scalar.activation(
                        out=osb,
                        in_=ps,
                        func=ACT_IDENT,
                        bias=bsb[:, ob : ob + 1],
                        scale=1.0,
                    )
                else:
                    # final block: drain on the (idle) DVE instead of queueing
                    # behind ScalarE -- shaves the post-train tail
                    drain.tensor_scalar_add(osb, ps, bsb[:, ob : ob + 1])
                # output DMA on the Sync HWDGE: ScalarE stays free for the
                # PSUM-draining activations
                nc.sync.dma_start(
                    out=out[ob * 128 : (ob + 1) * 128, r0 : r0 + nrows, :],
                    in_=osb,
                )

            # full ob=0 pass, then full ob=1 pass: the ob=1 weights are far
            # off the startup critical path.  Each pass starts and ends with
            # a 2-row block: the first needs only rows 0-3, the last
            # shortens the kernel tail (final drain covers 2 rows).
            for ob in range(2):
                conv_block(0, 2, ob, f"0_{ob}")
                for rb in range(2, HSH - RB - 1, RB):
                    conv_block(rb, RB, ob, f"{rb}_{ob}")
                conv_block(
                    HSH - 2, 2, ob, f"{HSH - 2}_{ob}",
                    drain=nc.vector if ob == 1 else nc.scalar,
                )

    nc.compile()
    return nc


def _get_nc():
    if "nc" not in _CACHE:
        _CACHE["nc"] = build_nc()
    return _CACHE["nc"]


def make_in_maps(x, depth, weight, bias):
    x = np.asarray(x, np.float32)
    depth = np.asarray(depth, np.float32)
    weight = np.asarray(weight, np.float32)
    bias = np.asarray(bias, np.float32)
    # (O, C, 3, 3) -> (C, ob, tap=i*3+j, o) with o = local index in the
    # 128-wide out-channel half ob
    wt9 = np.ascontiguousarray(
        np.transpose(
            weight.reshape(2, O // 2, C, 3, 3), (2, 0, 3, 4, 1)
        ).reshape(C, 2, 9, O // 2)
    ).astype(NPBF16)
    bb = np.ascontiguousarray(bias.reshape(2, O // 2).T)
    xb = x.astype(NPBF16)
    db = depth.astype(NPBF16)
    in_maps = []
    for cid in range(NCORES):
        n, hh = divmod(cid, 2)
        xsh = np.zeros((C, HIN, W), NPBF16)
        dsh = np.zeros((HIN, W), NPBF16)
        if hh == 0:
            xsh[:, 1:] = xb[n, :, : HSH + 1]
            dsh[1:] = db[n, 0, : HSH + 1]
        else:
            xsh[:, :-1] = xb[n, :, HSH - 1 :]
            dsh[:-1] = db[n, 0, HSH - 1 :]
        # depth replicated across the channel dim host-side: a device
        # partition-broadcast is far slower than shipping the copies.
        # rows 0-11 as separate x / depth; rows 12-65 packed [x|depth].
        dfull = np.broadcast_to(dsh[None], (C, HIN, W))
        xdp = np.empty((C, HIN - NSPLIT, 2, W), NPBF16)
        xdp[:, :, 0] = xsh[:, NSPLIT:]
        xdp[:, :, 1] = dfull[:, NSPLIT:]
        in_maps.append(
            {
                "xs": np.ascontiguousarray(xsh[:, :NSPLIT]),
                "dep": np.ascontiguousarray(dfull[:, :NSPLIT]),
                "xd": xdp,
                "wt": wt9,
                "bb": bb,
            }
        )
    return in_maps


def gather_out(results):
    out = np.empty((N, O, H, W), np.float32)
    for cid in range(NCORES):
        n, hh = divmod(cid, 2)
        out[n, :, hh * HSH : (hh + 1) * HSH] = results[cid]["out"].astype(np.float32)
    return out


def kernel(x, depth, camera_params, weight, bias):
    nc = _get_nc()
    in_maps = make_in_maps(x, depth, weight, bias)
    res = run_bass_kernel_spmd(nc, in_maps, list(range(NCORES)))
    return gather_out(res.results)


# revision 31
# speedup vs baseline: 1.0216x; 1.0216x over previous
"""DepthAwareConv2d Trainium2 kernel (bf16 v12).

Math: the reference's depth-modulated im2col GEMM is exactly
    out = conv2d(x * depth, weight, stride=1, pad=1) + bias
(depth broadcasts over channels; unfold(x)*unfold(depth) = unfold(x*depth)).

Sharding (8 cores): data-parallel over N (4 images) x spatial-parallel over
image row halves.  Core cid handles n = cid//2, row half = cid%2 (output rows
[0,64) or [64,128)), computing all 256 output channels for its half.  The
host ships each core its 64 input rows plus one halo/zero row on each side
(66 rows total), so the device program is identical on every core (SPMD) and
no collectives are needed.

The matmul train runs at the bf16 PE floor (217ns per 512-col matmul,
~63us for the 306 matmuls); everything else is edge-engineering driven by
trace measurements:
  * HW-measured HWDGE queue rates: Sync ~170GB/s, ScalarE ~95GB/s (it
    shares bandwidth with the activation-table load); startup is also
    HBM-contended by all 8 cores streaming at once.  The input schedule is
    laid out against those rates in consumption order (queues are FIFO):
    rows 0-11 ship as separate x / depth transfers so the first multiply
    only waits for two parallel 128KB queue heads; rows 12-65 ship packed
    [x|depth] per row (one DMA + ONE completion semaphore per chunk),
    distributed across both queues so each stays ahead of the train.
  * the bias DMA (128 partitions x 8B = 128 tiny descriptors, ~1-2us of
    queue time) is kept behind the early chunks.
  * NWARM short (128-col) zero matmuls run back-to-back from right after
    the engine barrier until the first real data lands: the PE's HAM
    activity window stays continuously busy, so the clock un-gates 4/8 ->
    8/8 ~3.4us after the barrier instead of mid-train (a gap during the
    ramp was observed to cost ~2-3us of half-clock matmuls); each adds only
    ~110ns of queue delay once real data is ready.
  * output DMA triggers ride the Sync HWDGE (ScalarE keeps draining PSUM
    via ACTIVATE); both ob passes run back to back (ob=1 weights are far
    off the critical path) and start/end with a 2-row block (early first
    matmul, short final drain); the final block drains on the idle DVE.

Per-core device kernel:
  1. DMA x/depth rows into SBUF; DVE-multiply into column-padded bf16 ypad
     (128 part, 66 x 130).
  2. Shift-conv: per output-row block and 128-wide out-channel half, 9
     accumulating bf16 matmuls (stationary = 128x128 weight tap, moving =
     shifted row window, free dim <= 512 = one PSUM bank) into one PSUM bank.
  3. ScalarE Identity(+bias) PSUM->SBUF (bf16), DMA out via Sync HWDGE.
"""

import ml_dtypes
import numpy as np

import concourse.bass as bass
import concourse.mybir as mybir
import concourse.tile as tile
from concourse import bacc
from concourse.bass_utils import run_bass_kernel_spmd

N, C, O, H, W = 4, 128, 256, 128, 128
HSH = H // 2  # output rows per core
HIN = HSH + 2  # input rows per core incl. halo/zero rows
NSPLIT = 12  # rows shipped as separate x / depth (the startup window)
NCORES = 8
F32 = mybir.dt.float32
BF16 = mybir.dt.bfloat16
ACT_IDENT = mybir.ActivationFunctionType.Identity
NPBF16 = ml_dtypes.bfloat16

RB = 4  # output rows per matmul tile (free dim RB*W = 512, one PSUM bank)
# packed [x|depth] chunks for rows 12-65: (row0, row1, queue); boundaries
# 0 mod 4; issue (= queue FIFO) order is consumption order per queue
PACKED = (
    (12, 20, "S"),
    (20, 28, "S"),
    (28, 36, "A"),  # the slow ScalarE queue gets one mid chunk + wt1
    (36, 44, "S"),
    (44, 56, "S"),
    (56, 66, "S"),
)
# 46 x ~107ns covers the barrier -> first-data window even on runs where
# 8-core HBM contention pushes the first chunk out late (observed bimodal
# arrival): a >1us PE gap during the ramp was seen to delay the HAM
# un-throttle mid-train, costing ~2.5us of half-clock matmuls.  Gating the
# first matmul on the whole ob=0 weight half also self-synchronizes the
# schedule: by train start the Sync queue is ~700KB deep, so every later
# chunk stays ahead of consumption regardless of HBM-contention luck
# (drip-feeding the weight taps to start ~1.4us earlier was measured to
# repay that lead as mid-train stalls on slow-DMA runs).
NWARM = 46

_CACHE = {}


def build_nc():
    nc = bacc.Bacc("TRN2", target_bir_lowering=False, debug=False, num_devices=NCORES)
    xs = nc.declare_dram_parameter("xs", [C, NSPLIT, W], BF16, isOutput=False)
    dep = nc.declare_dram_parameter("dep", [C, NSPLIT, W], BF16, isOutput=False)
    xd = nc.declare_dram_parameter(
        "xd", [C, HIN - NSPLIT, 2, W], BF16, isOutput=False
    )
    wt = nc.declare_dram_parameter("wt", [C, 2, 9, O // 2], BF16, isOutput=False)
    bb = nc.declare_dram_parameter("bb", [O // 2, 2], F32, isOutput=False)
    out = nc.declare_dram_parameter("out", [O, HSH, W], BF16, isOutput=True)

    with tile.TileContext(nc) as tc:
        with (
            tc.tile_pool(name="big", bufs=1) as big,
            tc.tile_pool(name="wp", bufs=1) as wp,
            tc.tile_pool(name="ch", bufs=7) as chp,
            tc.tile_pool(name="op", bufs=4) as op,
            tc.tile_pool(name="pp", bufs=8, space="PSUM") as pp,
        ):
            ypad = big.tile([C, HIN, W + 2], BF16)
            wsb = wp.tile([C, 2, 9, O // 2], BF16)
            bsb = wp.tile([O // 2, 2], F32)  # bsb[p, ob] = bias[ob*128 + p]
            ztile = wp.tile([C, 128], BF16)

            # zeros: warmup operand + ypad column borders.  GpSimd is idle
            # otherwise and starts right after the engine barrier, so the
            # warm-up matmuls (which read ztile) issue as early as possible.
            nc.gpsimd.memset(ztile, 0.0)
            nc.gpsimd.memset(ypad[:, :, 0], 0.0)
            nc.gpsimd.memset(ypad[:, :, W + 1], 0.0)

            warm = pp.tile([O // 2, 128], F32, tag="ps")
            for _ in range(NWARM):
                nc.tensor.matmul(warm, ztile, ztile, start=True, stop=True)

            split_tiles = {}
            packed_tiles = {}

            def split_dma(r0, r1):
                xb = chp.tile([C, 8, W], BF16, tag="xb", name=f"xb{r0}")
                db = chp.tile([C, 8, W], BF16, tag="db", name=f"db{r0}")
                nc.sync.dma_start(out=xb[:, : r1 - r0], in_=xs[:, r0:r1])
                nc.scalar.dma_start(out=db[:, : r1 - r0], in_=dep[:, r0:r1])
                split_tiles[r0] = (xb, db, r0, r1)

            def packed_dma(pi):
                r0, r1, q = PACKED[pi]
                xb = chp.tile([C, 12, 2, W], BF16, tag="xd", name=f"xd{r0}")
                eng = nc.sync if q == "S" else nc.scalar
                eng.dma_start(
                    out=xb[:, : r1 - r0], in_=xd[:, r0 - NSPLIT : r1 - NSPLIT]
                )
                packed_tiles[pi] = xb

            def mul_rows(r0, r1):
                # 4-aligned multiply blocks keep Tile's (quantized)
                # range-overlap check from dragging in a writer one byte
                # past the true read range of a conv block.
                if r0 < NSPLIT:
                    xb, db, base, end = next(
                        v for v in split_tiles.values() if v[2] <= r0 < v[3]
                    )
                    lo = r0 - base
                    in0 = xb[:, lo : lo + (r1 - r0)]
                    in1 = db[:, lo : lo + (r1 - r0)]
                else:
                    pi = next(
                        i for i, (a, b, _) in enumerate(PACKED) if a <= r0 < b
                    )
                    xb = packed_tiles[pi]
                    lo = r0 - PACKED[pi][0]
                    in0 = xb[:, lo : lo + (r1 - r0), 0]
                    in1 = xb[:, lo : lo + (r1 - r0), 1]
                nc.vector.tensor_mul(out=ypad[:, r0:r1, 1 : W + 1], in0=in0, in1=in1)

            # trigger order = consumption order per queue (FIFO): Sync gets
            # x0, the ob=0 weights, x1, then its packed chunks; ScalarE gets
            # dep0, dep1, bias, its packed chunk, the ob=1 weights.
            split_dma(0, 4)
            nc.sync.dma_start(out=wsb[:, 0], in_=wt.ap()[:, 0])
            split_dma(4, 12)
            nc.scalar.dma_start(out=bsb, in_=bb.ap())
            mul_rows(0, 4)
            for pi in range(len(PACKED)):
                packed_dma(pi)
            nc.scalar.dma_start(out=wsb[:, 1], in_=wt.ap()[:, 1])
            r = 4
            while r < HIN:
                r1 = min(r + 4, HIN)
                mul_rows(r, r1)
                r = r1

            def conv_block(r0, nrows, ob, name, drain=nc.scalar):
                # one accumulation group: rows [r0, r0+nrows), out-half ob
                ps = pp.tile([O // 2, nrows, W], F32, tag="ps", name=f"ps{name}")
                for p in range(9):
                    i, j = divmod(p, 3)
                    nc.tensor.matmul(
                        ps,
                        wsb[:, ob, p],
                        ypad[:, r0 + i : r0 + i + nrows, j : j + W],
                        start=(p == 0),
                        stop=(p == 8),
                    )
                osb = op.tile([O // 2, nrows, W], BF16, tag="osb", name=f"osb{name}")
                if drain is nc.scalar:
                    nc.scalar.activation(
                        out=osb,
                        in_=ps,
                        func=ACT_IDENT,
                        bias=bsb[:, ob : ob + 1],
                        scale=1.0,
                    )
                else:
                    # final block: drain on the (idle) DVE instead of queueing
                    # behind ScalarE -- shaves the post-train tail
                    drain.tensor_scalar_add(osb, ps, bsb[:, ob : ob + 1])
                # output DMA on the Sync HWDGE: ScalarE stays free for the
                # PSUM-draining activations
                nc.sync.dma_start(
                    out=out[ob * 128 : (ob + 1) * 128, r0 : r0 + nrows, :],
                    in_=osb,
                )

            # full ob=0 pass, then full ob=1 pass: the ob=1 weights are far
            # off the startup critical path.  Each pass starts and ends with
            # a 2-row block: the first needs only rows 0-3, the last
            # shortens the kernel tail (final drain covers 2 rows).
            for ob in range(2):
                conv_block(0, 2, ob, f"0_{ob}")
                for rb in range(2, HSH - RB - 1, RB):
                    conv_block(rb, RB, ob, f"{rb}_{ob}")
                conv_block(
                    HSH - 2, 2, ob, f"{HSH - 2}_{ob}",
                    drain=nc.vector if ob == 1 else nc.scalar,
                )

    nc.compile()
    return nc


def _get_nc():
    if "nc" not in _CACHE:
        _CACHE["nc"] = build_nc()
    return _CACHE["nc"]


def make_in_maps(x, depth, weight, bias):
    x = np.asarray(x, np.float32)
    depth = np.asarray(depth, np.float32)
    weight = np.asarray(weight, np.float32)
    bias = np.asarray(bias, np.float32)
    # (O, C, 3, 3) -> (C, ob, tap=i*3+j, o) with o = local index in the
    # 128-wide out-channel half ob
    wt9 = np.ascontiguousarray(
        np.transpose(
            weight.reshape(2, O // 2, C, 3, 3), (2, 0, 3, 4, 1)
        ).reshape(C, 2, 9, O // 2)
    ).astype(NPBF16)
    bb = np.ascontiguousarray(bias.reshape(2, O // 2).T)
    xb = x.astype(NPBF16)
    db = depth.astype(NPBF16)
    in_maps = []
    for cid in range(NCORES):
        n, hh = divmod(cid, 2)
        xsh = np.zeros((C, HIN, W), NPBF16)
        dsh = np.zeros((HIN, W), NPBF16)
        if hh == 0:
            xsh[:, 1:] = xb[n, :, : HSH + 1]
            dsh[1:] = db[n, 0, : HSH + 1]
        else:
            xsh[:, :-1] = xb[n, :, HSH - 1 :]
            dsh[:-1] = db[n, 0, HSH - 1 :]
        # depth replicated across the channel dim host-side: a device
        # partition-broadcast is far slower than shipping the copies.
        # rows 0-11 as separate x / depth; rows 12-65 packed [x|depth].
        dfull = np.broadcast_to(dsh[None], (C, HIN, W))
        xdp = np.empty((C, HIN - NSPLIT, 2, W), NPBF16)
        xdp[:, :, 0] = xsh[:, NSPLIT:]
        xdp[:, :, 1] = dfull[:, NSPLIT:]
        in_maps.append(
            {
                "xs": np.ascontiguousarray(xsh[:, :NSPLIT]),
                "dep": np.ascontiguousarray(dfull[:, :NSPLIT]),
                "xd": xdp,
                "wt": wt9,
                "bb": bb,
            }
        )
    return in_maps


def gather_out(results):
    out = np.empty((N, O, H, W), np.float32)
    for cid in range(NCORES):
        n, hh = divmod(cid, 2)
        out[n, :, hh * HSH : (hh + 1) * HSH] = results[cid]["out"].astype(np.float32)
    return out


def kernel(x, depth, camera_params, weight, bias):
    nc = _get_nc()
    in_maps = make_in_maps(x, depth, weight, bias)
    res = run_bass_kernel_spmd(nc, in_maps, list(range(NCORES)))
    return gather_out(res.results)


# revision 33
# speedup vs baseline: 1.0268x; 1.0051x over previous
"""DepthAwareConv2d Trainium2 kernel (bf16 v12).

Math: the reference's depth-modulated im2col GEMM is exactly
    out = conv2d(x * depth, weight, stride=1, pad=1) + bias
(depth broadcasts over channels; unfold(x)*unfold(depth) = unfold(x*depth)).

Sharding (8 cores): data-parallel over N (4 images) x spatial-parallel over
image row halves.  Core cid handles n = cid//2, row half = cid%2 (output rows
[0,64) or [64,128)), computing all 256 output channels for its half.  The
host ships each core its 64 input rows plus one halo/zero row on each side
(66 rows total), so the device program is identical on every core (SPMD) and
no collectives are needed.

The matmul train runs at the bf16 PE floor (217ns per 512-col matmul,
~63us for the 306 matmuls); everything else is edge-engineering driven by
trace measurements:
  * HW-measured HWDGE queue rates: Sync ~170GB/s, ScalarE ~95GB/s (it
    shares bandwidth with the activation-table load); startup is also
    HBM-contended by all 8 cores streaming at once.  The input schedule is
    laid out against those rates in consumption order (queues are FIFO):
    rows 0-11 ship as separate x / depth transfers so the first multiply
    only waits for two parallel 128KB queue heads; rows 12-65 ship packed
    [x|depth] per row (one DMA + ONE completion semaphore per chunk),
    distributed across both queues so each stays ahead of the train.
  * the bias DMA (128 partitions x 8B = 128 tiny descriptors, ~1-2us of
    queue time) is kept behind the early chunks.
  * NWARM short (128-col) zero matmuls run back-to-back from right after
    the engine barrier until the first real data lands: the PE's HAM
    activity window stays continuously busy, so the clock un-gates 4/8 ->
    8/8 ~3.4us after the barrier instead of mid-train (a gap during the
    ramp was observed to cost ~2-3us of half-clock matmuls); each adds only
    ~110ns of queue delay once real data is ready.
  * output DMA triggers ride the Sync HWDGE (ScalarE keeps draining PSUM
    via ACTIVATE); both ob passes run back to back (ob=1 weights are far
    off the critical path) and start/end with a 2-row block (early first
    matmul, short final drain); the final block drains on the idle DVE.

Per-core device kernel:
  1. DMA x/depth rows into SBUF; DVE-multiply into column-padded bf16 ypad
     (128 part, 66 x 130).
  2. Shift-conv: per output-row block and 128-wide out-channel half, 9
     accumulating bf16 matmuls (stationary = 128x128 weight tap, moving =
     shifted row window, free dim <= 512 = one PSUM bank) into one PSUM bank.
  3. ScalarE Identity(+bias) PSUM->SBUF (bf16), DMA out via Sync HWDGE.
"""

import ml_dtypes
import numpy as np

import concourse.bass as bass
import concourse.mybir as mybir
import concourse.tile as tile
from concourse import bacc
from concourse.bass_utils import run_bass_kernel_spmd

N, C, O, H, W = 4, 128, 256, 128, 128
HSH = H // 2  # output rows per core
HIN = HSH + 2  # input rows per core incl. halo/zero rows
NSPLIT = 12  # rows shipped as separate x / depth (the startup window)
NCORES = 8
F32 = mybir.dt.float32
BF16 = mybir.dt.bfloat16
ACT_IDENT = mybir.ActivationFunctionType.Identity
NPBF16 = ml_dtypes.bfloat16

RB = 4  # output rows per matmul tile (free dim RB*W = 512, one PSUM bank)
# packed [x|depth] chunks for rows 12-65: (row0, row1, queue); boundaries
# 0 mod 4; issue (= queue FIFO) order is consumption order per queue
PACKED = (
    (12, 20, "S"),
    (20, 28, "S"),
    (28, 36, "A"),  # the slow ScalarE queue gets one mid chunk + wt1
    (36, 44, "S"),
    (44, 56, "S"),
    (56, 66, "S"),
)
# 46 x ~107ns covers the barrier -> first-data window even on runs where
# 8-core HBM contention pushes the first chunk out late (observed bimodal
# arrival): a >1us PE gap during the ramp was seen to delay the HAM
# un-throttle mid-train, costing ~2.5us of half-clock matmuls.  Gating the
# first matmul on the whole ob=0 weight half also self-synchronizes the
# schedule: by train start the Sync queue is ~700KB deep, so every later
# chunk stays ahead of consumption regardless of HBM-contention luck
# (drip-feeding the weight taps to start ~1.4us earlier was measured to
# repay that lead as mid-train stalls on slow-DMA runs).
NWARM = 46

_CACHE = {}


def build_nc():
    nc = bacc.Bacc("TRN2", target_bir_lowering=False, debug=False, num_devices=NCORES)
    xs = nc.declare_dram_parameter("xs", [C, NSPLIT, W], BF16, isOutput=False)
    dep = nc.declare_dram_parameter("dep", [C, NSPLIT, W], BF16, isOutput=False)
    xd = nc.declare_dram_parameter(
        "xd", [C, HIN - NSPLIT, 2, W], BF16, isOutput=False
    )
    wt = nc.declare_dram_parameter("wt", [C, 2, 9, O // 2], BF16, isOutput=False)
    bb = nc.declare_dram_parameter("bb", [O // 2, 2], F32, isOutput=False)
    out = nc.declare_dram_parameter("out", [O, HSH, W], BF16, isOutput=True)

    with tile.TileContext(nc) as tc:
        with (
            tc.tile_pool(name="big", bufs=1) as big,
            tc.tile_pool(name="wp", bufs=1) as wp,
            tc.tile_pool(name="ch", bufs=7) as chp,
            tc.tile_pool(name="op", bufs=4) as op,
            tc.tile_pool(name="pp", bufs=8, space="PSUM") as pp,
        ):
            ypad = big.tile([C, HIN, W + 2], BF16)
            wsb = wp.tile([C, 2, 9, O // 2], BF16)
            bsb = wp.tile([O // 2, 2], F32)  # bsb[p, ob] = bias[ob*128 + p]
            ztile = wp.tile([C, 128], BF16)

            # zeros: warmup operand + ypad column borders.  GpSimd is idle
            # otherwise and starts right after the engine barrier, so the
            # warm-up matmuls (which read ztile) issue as early as possible.
            nc.gpsimd.memset(ztile, 0.0)
            # the ob=0 weight half rides the GpSimd SWDGE queue: a third
            # input stream in parallel with the two HWDGE queues, so the
            # first matmul's gate is max(x0+dep0+multiply, this transfer)
            # instead of x0+weights serialized on one queue
            nc.gpsimd.dma_start(out=wsb[:, 0], in_=wt.ap()[:, 0])
            nc.gpsimd.memset(ypad[:, :, 0], 0.0)
            nc.gpsimd.memset(ypad[:, :, W + 1], 0.0)

            warm = pp.tile([O // 2, 128], F32, tag="ps")
            for _ in range(NWARM):
                nc.tensor.matmul(warm, ztile, ztile, start=True, stop=True)

            split_tiles = {}
            packed_tiles = {}

            def split_dma(r0, r1):
                xb = chp.tile([C, 8, W], BF16, tag="xb", name=f"xb{r0}")
                db = chp.tile([C, 8, W], BF16, tag="db", name=f"db{r0}")
                nc.sync.dma_start(out=xb[:, : r1 - r0], in_=xs[:, r0:r1])
                nc.scalar.dma_start(out=db[:, : r1 - r0], in_=dep[:, r0:r1])
                split_tiles[r0] = (xb, db, r0, r1)

            def packed_dma(pi):
                r0, r1, q = PACKED[pi]
                xb = chp.tile([C, 12, 2, W], BF16, tag="xd", name=f"xd{r0}")
                eng = nc.sync if q == "S" else nc.scalar
                eng.dma_start(
                    out=xb[:, : r1 - r0], in_=xd[:, r0 - NSPLIT : r1 - NSPLIT]
                )
                packed_tiles[pi] = xb

            def mul_rows(r0, r1):
                # 4-aligned multiply blocks keep Tile's (quantized)
                # range-overlap check from dragging in a writer one byte
                # past the true read range of a conv block.
                if r0 < NSPLIT:
                    xb, db, base, end = next(
                        v for v in split_tiles.values() if v[2] <= r0 < v[3]
                    )
                    lo = r0 - base
                    in0 = xb[:, lo : lo + (r1 - r0)]
                    in1 = db[:, lo : lo + (r1 - r0)]
                else:
                    pi = next(
                        i for i, (a, b, _) in enumerate(PACKED) if a <= r0 < b
                    )
                    xb = packed_tiles[pi]
                    lo = r0 - PACKED[pi][0]
                    in0 = xb[:, lo : lo + (r1 - r0), 0]
                    in1 = xb[:, lo : lo + (r1 - r0), 1]
                nc.vector.tensor_mul(out=ypad[:, r0:r1, 1 : W + 1], in0=in0, in1=in1)

            # trigger order = consumption order per queue (FIFO): Sync gets
            # x0, the ob=0 weights, x1, then its packed chunks; ScalarE gets
            # dep0, dep1, bias, its packed chunk, the ob=1 weights.
            split_dma(0, 4)
            split_dma(4, 12)
            nc.scalar.dma_start(out=bsb, in_=bb.ap())
            mul_rows(0, 4)
            for pi in range(len(PACKED)):
                packed_dma(pi)
            nc.scalar.dma_start(out=wsb[:, 1], in_=wt.ap()[:, 1])
            r = 4
            while r < HIN:
                r1 = min(r + 4, HIN)
                mul_rows(r, r1)
                r = r1

            def conv_block(r0, nrows, ob, name, drain=nc.scalar):
                # one accumulation group: rows [r0, r0+nrows), out-half ob
                ps = pp.tile([O // 2, nrows, W], F32, tag="ps", name=f"ps{name}")
                for p in range(9):
                    i, j = divmod(p, 3)
                    nc.tensor.matmul(
                        ps,
                        wsb[:, ob, p],
                        ypad[:, r0 + i : r0 + i + nrows, j : j + W],
                        start=(p == 0),
                        stop=(p == 8),
                    )
                osb = op.tile([O // 2, nrows, W], BF16, tag="osb", name=f"osb{name}")
                if drain is nc.scalar:
                    nc.scalar.activation(
                        out=osb,
                        in_=ps,
                        func=ACT_IDENT,
                        bias=bsb[:, ob : ob + 1],
                        scale=1.0,
                    )
                else:
                    # final block: drain on the (idle) DVE instead of queueing
                    # behind ScalarE -- shaves the post-train tail
                    drain.tensor_scalar_add(osb, ps, bsb[:, ob : ob + 1])
                # output DMA on the Sync HWDGE: ScalarE stays free for the
                # PSUM-draining activations
                nc.sync.dma_start(
                    out=out[ob * 128 : (ob + 1) * 128, r0 : r0 + nrows, :],
                    in_=osb,
                )

            # full ob=0 pass, then full ob=1 pass: the ob=1 weights are far
            # off the startup critical path.  Each pass starts and ends with
            # a 2-row block: the first needs only rows 0-3, the last
            # shortens the kernel tail (final drain covers 2 rows).
            for ob in range(2):
                conv_block(0, 2, ob, f"0_{ob}")
                for rb in range(2, HSH - RB - 1, RB):
                    conv_block(rb, RB, ob, f"{rb}_{ob}")
                conv_block(
                    HSH - 2, 2, ob, f"{HSH - 2}_{ob}",
                    drain=nc.vector if ob == 1 else nc.scalar,
                )

    nc.compile()
    return nc


def _get_nc():
    if "nc" not in _CACHE:
        _CACHE["nc"] = build_nc()
    return _CACHE["nc"]


def make_in_maps(x, depth, weight, bias):
    x = np.asarray(x, np.float32)
    depth = np.asarray(depth, np.float32)
    weight = np.asarray(weight, np.float32)
    bias = np.asarray(bias, np.float32)
    # (O, C, 3, 3) -> (C, ob, tap=i*3+j, o) with o = local index in the
    # 128-wide out-channel half ob
    wt9 = np.ascontiguousarray(
        np.transpose(
            weight.reshape(2, O // 2, C, 3, 3), (2, 0, 3, 4, 1)
        ).reshape(C, 2, 9, O // 2)
    ).astype(NPBF16)
    bb = np.ascontiguousarray(bias.reshape(2, O // 2).T)
    xb = x.astype(NPBF16)
    db = depth.astype(NPBF16)
    in_maps = []
    for cid in range(NCORES):
        n, hh = divmod(cid, 2)
        xsh = np.zeros((C, HIN, W), NPBF16)
        dsh = np.zeros((HIN, W), NPBF16)
        if hh == 0:
            xsh[:, 1:] = xb[n, :, : HSH + 1]
            dsh[1:] = db[n, 0, : HSH + 1]
        else:
            xsh[:, :-1] = xb[n, :, HSH - 1 :]
            dsh[:-1] = db[n, 0, HSH - 1 :]
        # depth replicated across the channel dim host-side: a device
        # partition-broadcast is far slower than shipping the copies.
        # rows 0-11 as separate x / depth; rows 12-65 packed [x|depth].
        dfull = np.broadcast_to(dsh[None], (C, HIN, W))
        xdp = np.empty((C, HIN - NSPLIT, 2, W), NPBF16)
        xdp[:, :, 0] = xsh[:, NSPLIT:]
        xdp[:, :, 1] = dfull[:, NSPLIT:]
        in_maps.append(
            {
                "xs": np.ascontiguousarray(xsh[:, :NSPLIT]),
                "dep": np.ascontiguousarray(dfull[:, :NSPLIT]),
                "xd": xdp,
                "wt": wt9,
                "bb": bb,
            }
        )
    return in_maps


def gather_out(results):
    out = np.empty((N, O, H, W), np.float32)
    for cid in range(NCORES):
        n, hh = divmod(cid, 2)
        out[n, :, hh * HSH : (hh + 1) * HSH] = results[cid]["out"].astype(np.float32)
    return out


def kernel(x, depth, camera_params, weight, bias):
    nc = _get_nc()
    in_maps = make_in_maps(x, depth, weight, bias)
    res = run_bass_kernel_spmd(nc, in_maps, list(range(NCORES)))
    return gather_out(res.results)
